# revision 1
# baseline (speedup 1.0000x reference)
"""GCN block (GCNConv + BN(eval) + ReLU) on 8 Trainium2 NeuronCores.

Strategy (fully data-parallel, no collectives):
  out = relu(BN(D^{-1/2}(A+I)D^{-1/2} (x W) + b))
      = relu(dis_dst * ((sum_{e->dst} xs[src] + xs[dst]) @ W') + b')
  where xs = x * dis (dis = deg^{-1/2}), W' = W * s, b' = b*s + t (BN folded).

  Nodes are sharded across 8 cores by destination block (degree-balanced
  snake deal).  The host pre-expands each core's edge source rows into a
  DENSE stream laid out exactly as the PE wants to consume it
  ([128 slots, group, feat], edge slots grouped per 128-dst tile), so the
  device reads it with plain sequential HWDGE dma_start -- no per-edge
  dma_gather and no GpSimd Q7 descriptor generation (which was the
  bottleneck: ~7us per 1024-row gather).

  Per 128-dst tile: one-hot selection matrices are shipped from the host
  in fp8e4 ({0,1} exact, half the bytes of bf16), upcast per tile to bf16
  by one DVE copy, edge slots are reduced into [feat, dst] PSUM via
  selection matmuls, the 512x512 transform GEMM + K=1 bias matmul follow,
  and ReLU (with the per-dst dis scale fused) writes bf16 output.
"""

import sys

if "/opt/trn_rl_repo" not in sys.path:
    sys.path.insert(0, "/opt/trn_rl_repo")

import math

import ml_dtypes
import numpy as np

BF16 = ml_dtypes.bfloat16
FP8E4 = ml_dtypes.float8_e4m3  # TRN FP8_EXP4 encoding (not OCP e4m3fn)

N_CORES = 8
P = 128
BN_EPS = 1e-5
TB = 6  # dst tiles per DMA batch


def _prep(x, edge_index, W, b, gamma, beta, running_mean, running_var):
    """Host-side preprocessing: sharding, edge slotting, dense stream
    expansion, BN folding.  Returns (meta, in_maps)."""
    N, F = x.shape
    F_OUT = W.shape[1]
    KC = F // P
    assert N % N_CORES == 0
    NB = N // N_CORES
    T = math.ceil(NB / P)  # dst tiles per core

    src = np.asarray(edge_index[0], dtype=np.int64)
    dst = np.asarray(edge_index[1], dtype=np.int64)

    deg = 1.0 + np.bincount(dst, minlength=N).astype(np.float64)
    dis = (1.0 / np.sqrt(deg)).astype(np.float32)

    xf = np.asarray(x, np.float32)
    # int8 quantization of x against a global scale; the per-row fp8
    # rounding of dis (which rides the selection matrix) is compensated
    # into the quantization step so only ~1% int8 noise remains.
    s_q = np.float32(4.75) * np.float32(xf.std()) / np.float32(127.0)
    dis8 = dis.astype(FP8E4)           # fp8(dis): exact value the sel carries
    dis8f = dis8.astype(np.float32)
    r_row = dis / dis8f                # in [1/(1+2^-4), 1+2^-4]
    x_q = np.clip(np.rint(xf * (r_row / s_q)[:, None]), -127, 127
                  ).astype(np.int8)    # [N, F]
    xs = (xf * (dis / s_q)[:, None]).astype(BF16)  # self-loop rows (pre /s_q)

    # BN folding; s_q (the int8 dequant step) folds into W as well since
    # both the edge stream and the self rows are pre-divided by s_q.
    s = (np.asarray(gamma, np.float32)
         / np.sqrt(np.asarray(running_var, np.float32) + BN_EPS))
    t = np.asarray(beta, np.float32) - np.asarray(running_mean, np.float32) * s
    Wp = (np.asarray(W, np.float32) * (s * s_q)[None, :]).astype(BF16)
    bp = (np.asarray(b, np.float32) * s + t).astype(np.float32)
    wp = np.ascontiguousarray(Wp.reshape(KC, P, F_OUT).transpose(1, 0, 2))

    # ---- degree-balanced node -> (core, tile, slot) assignment (snake deal)
    NBINS = N_CORES * T
    order = np.argsort(-(deg - 1.0), kind="stable")
    assign = np.empty(N, np.int64)   # node -> bin
    slot_of = np.empty(N, np.int64)  # node -> slot within bin
    pos = 0
    rnd = 0
    while pos < N:
        chunk = order[pos:pos + NBINS]
        if rnd % 2 == 0:
            bins = np.arange(len(chunk))
        else:
            bins = NBINS - 1 - np.arange(len(chunk))
        assign[chunk] = bins
        slot_of[chunk] = rnd
        pos += NBINS
        rnd += 1
    assert rnd <= P, f"too many slot rounds {rnd}"
    core_of_bin = assign % N_CORES
    tile_of_bin = assign // N_CORES

    # node_map[k][t, p] = original node id (or -1)
    node_map = np.full((N_CORES, T, P), -1, dtype=np.int64)
    node_map[core_of_bin, tile_of_bin, slot_of] = np.arange(N)

    e_core = core_of_bin[dst]
    e_tile = tile_of_bin[dst]
    e_slot = slot_of[dst]

    # ---- pass 1: per-core edge lists sorted by tile, per-tile counts
    per_core = []
    cnt = np.zeros((N_CORES, T), dtype=np.int64)
    for k in range(N_CORES):
        m = e_core == k
        s_k = src[m]
        t_k = e_tile[m]
        p_k = e_slot[m]
        o = np.argsort(t_k, kind="stable")
        s_k, t_k, p_k = s_k[o], t_k[o], p_k[o]
        bounds = np.searchsorted(t_k, np.arange(T + 1))
        cnt[k] = bounds[1:] - bounds[:-1]
        per_core.append((s_k, p_k, bounds))

    S_t = (np.ceil(cnt.max(axis=0) / P).astype(np.int64) * P)
    S_t = np.maximum(S_t, P)
    off_t = np.concatenate([[0], np.cumsum(S_t)])
    TOT = int(off_t[-1])
    NG_t = (S_t // P).astype(np.int64)
    G_off = (off_t // P).astype(np.int64)
    G_TOT = TOT // P

    # ---- pass 2: per-core arrays
    in_maps = []
    for k in range(N_CORES):
        s_k, p_k, bounds = per_core[k]
        srcs_flat = np.zeros(TOT, dtype=np.int64)
        dstl_flat = np.full(TOT, -1.0, dtype=np.float32)
        for tt in range(T):
            t_lo, t_hi = bounds[tt], bounds[tt + 1]
            n_e = t_hi - t_lo
            o = off_t[tt]
            srcs_flat[o:o + n_e] = s_k[t_lo:t_hi]
            dstl_flat[o:o + n_e] = p_k[t_lo:t_hi].astype(np.float32)
        # dense expanded stream: stream[p, g, :] = x_q[src of slot g*128+p]
        stream = np.ascontiguousarray(
            x_q[srcs_flat].reshape(G_TOT, P, F).transpose(1, 0, 2))
        # selection matrices, fp8: sel[p, g*128 + d] = fp8(dis_src) iff edge
        # slot g*128+p has dst slot d (0 otherwise / padding)
        oh = (dstl_flat[:, None] == np.arange(P, dtype=np.float32)[None, :])
        selval = dis8[srcs_flat]  # [TOT] fp8
        sel = np.ascontiguousarray(
            (oh * selval[:, None].astype(np.float32))
            .reshape(G_TOT, P, P).transpose(1, 0, 2).reshape(P, G_TOT * P)
        ).astype(FP8E4)

        nm = node_map[k]  # [T, P]
        valid = nm >= 0
        nm_safe = np.where(valid, nm, 0)
        dis_tp = np.where(valid, dis[nm_safe], 1.0).astype(np.float32)  # [T, P]
        dis_t = np.ascontiguousarray(dis_tp.T)  # [128, T]
        invdis = np.zeros((1, T * P), dtype=BF16)
        invdis[0, :] = np.where(valid, 1.0 / np.maximum(dis_tp, 1e-9), 0.0
                                ).reshape(-1).astype(BF16)
        xso_rows = np.where(valid[:, :, None], xs[nm_safe].astype(np.float32), 0.0)
        xs_own = np.ascontiguousarray(
            xso_rows.transpose(1, 0, 2)).astype(BF16)  # [128, T, F]
        ident = np.eye(P, dtype=np.float32).astype(BF16)
        in_maps.append({
            "xs_own": xs_own,
            "ident": np.ascontiguousarray(ident),
            "stream": stream,
            "sel": sel,
            "dis_t": dis_t,
            "invdis": invdis,
            "wp": wp,
            "bp": bp.reshape(1, F_OUT).astype(BF16),
        })

    meta = {
        "N": N, "F": F, "F_OUT": F_OUT, "KC": KC, "NB": NB, "T": T,
        "TOT": TOT, "G_TOT": G_TOT,
        "NG_t": NG_t.tolist(), "G_off": G_off.tolist(),
        "node_map": node_map,
    }
    return meta, in_maps


def _build_program(meta):
    """Emit the Bass/Tile program (shared by all cores)."""
    import concourse.bacc as bacc
    import concourse.mybir as mybir
    import concourse.tile as tile

    F, F_OUT, KC = meta["F"], meta["F_OUT"], meta["KC"]
    T, G_TOT = meta["T"], meta["G_TOT"]
    NG_t, G_off = meta["NG_t"], meta["G_off"]

    dt = mybir.dt
    nc = bacc.Bacc("TRN2", target_bir_lowering=False, debug=False,
                   enable_asserts=False, num_devices=N_CORES,
                   num_swdge_queues=4)

    stream = nc.dram_tensor("stream", [P, G_TOT, F], dt.int8, kind="ExternalInput").ap()
    sel = nc.dram_tensor("sel", [P, G_TOT * P], dt.float8e4, kind="ExternalInput").ap()
    dis_t = nc.dram_tensor("dis_t", [P, T], dt.float32, kind="ExternalInput").ap()
    invdis = nc.dram_tensor("invdis", [1, T * P], dt.bfloat16, kind="ExternalInput").ap()
    ident = nc.dram_tensor("ident", [P, P], dt.bfloat16, kind="ExternalInput").ap()
    xs_own = nc.dram_tensor("xs_own", [P, T, F], dt.bfloat16, kind="ExternalInput").ap()
    wp = nc.dram_tensor("wp", [P, KC, F_OUT], dt.bfloat16, kind="ExternalInput").ap()
    bp = nc.dram_tensor("bp", [1, F_OUT], dt.bfloat16, kind="ExternalInput").ap()
    out = nc.dram_tensor("out", [P, T, F_OUT], dt.bfloat16, kind="ExternalOutput").ap()

    max_ng = max(NG_t)
    max_bw = max(G_off[min(t0 + TB, T)] - G_off[t0] for t0 in range(0, T, TB))

    with tile.TileContext(nc) as tc:
        with (
            tc.tile_pool(name="const", bufs=1) as cpool,
            tc.tile_pool(name="gbuf", bufs=2) as gpool,
            tc.tile_pool(name="sel8", bufs=2) as s8pool,
            tc.tile_pool(name="xso", bufs=2) as xpool,
            tc.tile_pool(name="aggT", bufs=3) as aggpool,
            tc.tile_pool(name="outsb", bufs=2) as opool,
            tc.tile_pool(name="psA", bufs=3, space="PSUM") as psA,
            tc.tile_pool(name="psB", bufs=3, space="PSUM") as psB,
        ):
            # resident constants
            ident_sb = cpool.tile([P, P], dt.bfloat16, tag="ident")
            nc.sync.dma_start(ident_sb[:], ident[:])
            dis_sb = cpool.tile([P, T], dt.float32, tag="dis")
            nc.sync.dma_start(dis_sb[:], dis_t[:])
            invdis_sb = cpool.tile([1, T * P], dt.bfloat16, tag="invdis")
            nc.sync.dma_start(invdis_sb[:], invdis[:])
            wp_sb = cpool.tile([P, KC, F_OUT], dt.bfloat16, tag="wp")
            nc.sync.dma_start(wp_sb[:], wp[:])
            bp_sb = cpool.tile([1, F_OUT], dt.bfloat16, tag="bp")
            nc.sync.dma_start(bp_sb[:], bp[:])

            for t0 in range(0, T, TB):
                t1 = min(t0 + TB, T)
                nb_t = t1 - t0
                sg0, sg1 = G_off[t0], G_off[t1]

                g_sb = gpool.tile([P, max_bw, F], dt.bfloat16, tag="g")
                # inline int8 -> bf16 cast during the DMA (SWDGE only);
                # first batch split per tile so PE starts ~3x sooner
                if t0 == 0:
                    for t in range(t0, t1):
                        ga, gb = G_off[t] - sg0, G_off[t + 1] - sg0
                        nc.gpsimd.dma_start(g_sb[:, ga:gb, :],
                                            stream[:, sg0 + ga:sg0 + gb, :])
                else:
                    nc.gpsimd.dma_start(g_sb[:, :sg1 - sg0, :],
                                        stream[:, sg0:sg1, :])
                self8_sb = s8pool.tile([P, max_bw * P], dt.float8e4, tag="sel8")
                nc.sync.dma_start(self8_sb[:, :(sg1 - sg0) * P],
                                  sel[:, sg0 * P:sg1 * P])
                xso_sb = xpool.tile([P, TB, F], dt.bfloat16, tag="xso")
                nc.sync.dma_start(xso_sb[:, :nb_t, :], xs_own[:, t0:t1, :])
                out_blk = opool.tile([P, TB, F_OUT], dt.bfloat16, tag="out_sb")

                for t in range(t0, t1):
                    ng = NG_t[t]
                    goff = G_off[t] - sg0

                    # self-loop term: aggT[fchunk, dst] = xs_own_tile^T (rhs=I)
                    aggT_ps = psA.tile([P, F], dt.float32, tag="aggT_ps")
                    for c in range(KC):
                        nc.tensor.matmul(
                            aggT_ps[:, c * P:(c + 1) * P],
                            lhsT=xso_sb[:, t - t0, c * P:(c + 1) * P],
                            rhs=ident_sb[:],
                            start=(c == 0),
                            stop=False,
                            skip_group_check=True,
                        )
                    # selection matmuls: aggT[fchunk, dst] += G_chunk^T @ selR
                    # (rhs consumed directly in fp8 -- mixed-dtype matmul)
                    for g in range(ng):
                        for c in range(KC):
                            nc.tensor.matmul(
                                aggT_ps[:, c * P:(c + 1) * P],
                                lhsT=g_sb[:, goff + g, c * P:(c + 1) * P],
                                rhs=self8_sb[:, (goff + g) * P:(goff + g + 1) * P],
                                start=False,
                                stop=(g == ng - 1 and c == KC - 1),
                                skip_group_check=True,
                            )

                    aggT_sb = aggpool.tile([P, F], dt.bfloat16, tag="aggT_sb")
                    nc.vector.tensor_copy(aggT_sb[:], aggT_ps[:])

                    # transform GEMM + K=1 bias row (bias pre-scaled by 1/dis)
                    out_ps = psB.tile([P, F_OUT], dt.float32, tag="out_ps")
                    for c in range(KC):
                        nc.tensor.matmul(
                            out_ps[:],
                            lhsT=aggT_sb[:, c * P:(c + 1) * P],
                            rhs=wp_sb[:, c, :],
                            start=(c == 0),
                            stop=False,
                        )
                    nc.tensor.matmul(
                        out_ps[:],
                        lhsT=invdis_sb[:1, t * P:(t + 1) * P],
                        rhs=bp_sb[:1, :],
                        start=False,
                        stop=True,
                    )

                    nc.scalar.activation(
                        out_blk[:, t - t0, :],
                        out_ps[:],
                        mybir.ActivationFunctionType.Relu,
                        scale=dis_sb[:, t:t + 1],
                    )

                nc.sync.dma_start(out[:, t0:t1, :], out_blk[:, :nb_t, :])

    nc.compile()
    return nc


_CACHE = {}


def _get_program(meta):
    key = (meta["N"], meta["F"], meta["F_OUT"], meta["TOT"], meta["G_TOT"],
           tuple(meta["NG_t"]))
    if key not in _CACHE:
        _CACHE[key] = _build_program(meta)
    return _CACHE[key]


def kernel(x, edge_index, W, b, gamma, beta, running_mean, running_var,
           _want_results_holder=None, _run_kwargs=None):
    meta, in_maps = _prep(x, edge_index, W, b, gamma, beta,
                          running_mean, running_var)
    nc = _get_program(meta)

    from concourse.bass_utils import run_bass_kernel_spmd

    res = run_bass_kernel_spmd(nc, in_maps, core_ids=list(range(N_CORES)),
                               **(_run_kwargs or {}))
    if _want_results_holder is not None:
        _want_results_holder.append((nc, meta, in_maps, res))

    T, F_OUT = meta["T"], meta["F_OUT"]
    node_map = meta["node_map"]
    out = np.empty((meta["N"], F_OUT), dtype=np.float32)
    for k in range(N_CORES):
        tiled = np.asarray(res.results[k]["out"], dtype=np.float32)  # [128, T, F_OUT]
        rows = np.ascontiguousarray(tiled.transpose(1, 0, 2))  # [T, 128, F]
        nm = node_map[k]
        valid = nm >= 0
        out[nm[valid]] = rows[valid]
    return out



# revision 3
# speedup vs baseline: 1.2619x; 1.2619x over previous
"""GCN block (GCNConv + BN(eval) + ReLU) on 8 Trainium2 NeuronCores.

Strategy (fully data-parallel, no collectives):
  out = relu(BN(D^{-1/2}(A+I)D^{-1/2} (x W) + b))
      = relu(dis_dst * ((sum_{e->dst} xs[src] + xs[dst]) @ W') + b')
  where xs = x * dis (dis = deg^{-1/2}), W' = W * s, b' = b*s + t (BN folded).

  Nodes are sharded across 8 cores by destination block (degree-balanced
  snake deal).  The host pre-expands each core's edge source rows into a
  DENSE stream laid out exactly as the PE wants to consume it
  ([128 slots, group, feat], edge slots grouped per 128-dst tile), so the
  device reads it with plain sequential HWDGE dma_start.

  The stream is quantized to fp8 E3M4 (4 mantissa bits, ~1.2% end-to-end
  rel err vs the 2e-2 gate) so the PE consumes it directly at 1 byte per
  element: no SWDGE int8->bf16 inline-cast DMA (which paid the 2x write
  side on the DMA fabric and was the co-bottleneck at ~102MB of fabric
  traffic per core).  Selection matrices (one-hot * dis_src, rounding
  compensated into the stream rows per-source) are also E3M4.

  Per 128-dst tile: edge slots are reduced into [feat, dst] PSUM via
  selection matmuls, the self-loop rows are added by the DVE during the
  PSUM->SBUF copy (tensor_tensor add with a host-pre-transposed
  [feat, dst] layout -- no PE identity-transpose matmuls), the 512x512
  transform GEMM + K=1 bias matmul follow, and ReLU (with the per-dst
  dis scale fused) writes bf16 output.
"""

import sys

if "/opt/trn_rl_repo" not in sys.path:
    sys.path.insert(0, "/opt/trn_rl_repo")

import math

import ml_dtypes
import numpy as np

BF16 = ml_dtypes.bfloat16
FP8E3 = ml_dtypes.float8_e3m4  # TRN FP8_EXP3 (1-3-4)

N_CORES = 8
P = 128
BN_EPS = 1e-5
TB = 6  # dst tiles per DMA batch


def _prep(x, edge_index, W, b, gamma, beta, running_mean, running_var):
    """Host-side preprocessing: sharding, edge slotting, dense stream
    expansion, BN folding.  Returns (meta, in_maps)."""
    N, F = x.shape
    F_OUT = W.shape[1]
    KC = F // P
    assert N % N_CORES == 0
    NB = N // N_CORES
    T = math.ceil(NB / P)  # dst tiles per core

    src = np.asarray(edge_index[0], dtype=np.int64)
    dst = np.asarray(edge_index[1], dtype=np.int64)

    deg = 1.0 + np.bincount(dst, minlength=N).astype(np.float64)
    dis = (1.0 / np.sqrt(deg)).astype(np.float32)

    xf = np.asarray(x, np.float32)
    # fp8 e3m4 stream; the rounding of dis (which rides the selection
    # matrix in e3m4) is compensated into the stream quantization so the
    # carried product is dis exactly on average.
    dis8 = dis.astype(FP8E3)
    dis8f = dis8.astype(np.float32)
    r_row = dis / dis8f
    x8 = (xf * r_row[:, None]).astype(FP8E3)       # [N, F] stream source
    xs = (xf * dis[:, None]).astype(np.float32)    # self-loop rows (exact)

    # BN folding into W and b.
    s = (np.asarray(gamma, np.float32)
         / np.sqrt(np.asarray(running_var, np.float32) + BN_EPS))
    t = np.asarray(beta, np.float32) - np.asarray(running_mean, np.float32) * s
    Wp = (np.asarray(W, np.float32) * s[None, :]).astype(BF16)
    bp = (np.asarray(b, np.float32) * s + t).astype(np.float32)
    wp = np.ascontiguousarray(Wp.reshape(KC, P, F_OUT).transpose(1, 0, 2))

    # ---- degree-balanced node -> (core, tile, slot) assignment (snake deal)
    NBINS = N_CORES * T
    order = np.argsort(-(deg - 1.0), kind="stable")
    assign = np.empty(N, np.int64)   # node -> bin
    slot_of = np.empty(N, np.int64)  # node -> slot within bin
    pos = 0
    rnd = 0
    while pos < N:
        chunk = order[pos:pos + NBINS]
        if rnd % 2 == 0:
            bins = np.arange(len(chunk))
        else:
            bins = NBINS - 1 - np.arange(len(chunk))
        assign[chunk] = bins
        slot_of[chunk] = rnd
        pos += NBINS
        rnd += 1
    assert rnd <= P, f"too many slot rounds {rnd}"
    core_of_bin = assign % N_CORES
    tile_of_bin = assign // N_CORES

    # node_map[k][t, p] = original node id (or -1)
    node_map = np.full((N_CORES, T, P), -1, dtype=np.int64)
    node_map[core_of_bin, tile_of_bin, slot_of] = np.arange(N)

    e_core = core_of_bin[dst]
    e_tile = tile_of_bin[dst]
    e_slot = slot_of[dst]

    # ---- pass 1: per-core edge lists sorted by tile, per-tile counts
    per_core = []
    cnt = np.zeros((N_CORES, T), dtype=np.int64)
    for k in range(N_CORES):
        m = e_core == k
        s_k = src[m]
        t_k = e_tile[m]
        p_k = e_slot[m]
        o = np.argsort(t_k, kind="stable")
        s_k, t_k, p_k = s_k[o], t_k[o], p_k[o]
        bounds = np.searchsorted(t_k, np.arange(T + 1))
        cnt[k] = bounds[1:] - bounds[:-1]
        per_core.append((s_k, p_k, bounds))

    S_t = (np.ceil(cnt.max(axis=0) / P).astype(np.int64) * P)
    S_t = np.maximum(S_t, P)
    off_t = np.concatenate([[0], np.cumsum(S_t)])
    TOT = int(off_t[-1])
    NG_t = (S_t // P).astype(np.int64)
    G_off = (off_t // P).astype(np.int64)
    G_TOT = TOT // P

    # ---- pass 2: per-core arrays
    in_maps = []
    for k in range(N_CORES):
        s_k, p_k, bounds = per_core[k]
        srcs_flat = np.zeros(TOT, dtype=np.int64)
        dstl_flat = np.full(TOT, -1.0, dtype=np.float32)
        for tt in range(T):
            t_lo, t_hi = bounds[tt], bounds[tt + 1]
            n_e = t_hi - t_lo
            o = off_t[tt]
            srcs_flat[o:o + n_e] = s_k[t_lo:t_hi]
            dstl_flat[o:o + n_e] = p_k[t_lo:t_hi].astype(np.float32)
        # dense expanded stream: stream[p, g, :] = x8[src of slot g*128+p]
        stream = np.ascontiguousarray(
            x8[srcs_flat].reshape(G_TOT, P, F).transpose(1, 0, 2))
        # selection matrices, fp8 e3m4:
        # sel[p, g*128 + d] = fp8(dis_src) iff edge slot g*128+p has dst
        # slot d (0 otherwise / padding)
        oh = (dstl_flat[:, None] == np.arange(P, dtype=np.float32)[None, :])
        selval = dis8[srcs_flat]  # [TOT] fp8
        sel = np.ascontiguousarray(
            (oh * selval[:, None].astype(np.float32))
            .reshape(G_TOT, P, P).transpose(1, 0, 2).reshape(P, G_TOT * P)
        ).astype(FP8E3)

        nm = node_map[k]  # [T, P]
        valid = nm >= 0
        nm_safe = np.where(valid, nm, 0)
        dis_tp = np.where(valid, dis[nm_safe], 1.0).astype(np.float32)  # [T, P]
        dis_t = np.ascontiguousarray(dis_tp.T)  # [128, T]
        invdis = np.zeros((1, T * P), dtype=BF16)
        invdis[0, :] = np.where(valid, 1.0 / np.maximum(dis_tp, 1e-9), 0.0
                                ).reshape(-1).astype(BF16)
        # self-loop rows pre-transposed to the aggT layout:
        # xsoT[p, t, c*128 + d] = xs[node(t, d), c*128 + p]
        xso_rows = np.where(valid[:, :, None], xs[nm_safe], 0.0)  # [T, P(d), F]
        xsoT = np.ascontiguousarray(
            xso_rows.reshape(T, P, KC, P).transpose(3, 0, 2, 1)
            .reshape(P, T, KC * P)).astype(BF16)
        in_maps.append({
            "stream": stream,
            "sel": sel,
            "dis_t": dis_t,
            "invdis": invdis,
            "xsoT": xsoT,
            "wp": wp,
            "bp": bp.reshape(1, F_OUT).astype(BF16),
        })

    meta = {
        "N": N, "F": F, "F_OUT": F_OUT, "KC": KC, "NB": NB, "T": T,
        "TOT": TOT, "G_TOT": G_TOT,
        "NG_t": NG_t.tolist(), "G_off": G_off.tolist(),
        "node_map": node_map,
    }
    return meta, in_maps


def _build_program(meta):
    """Emit the Bass/Tile program (shared by all cores)."""
    import concourse.bacc as bacc
    import concourse.mybir as mybir
    import concourse.tile as tile

    F, F_OUT, KC = meta["F"], meta["F_OUT"], meta["KC"]
    T, G_TOT = meta["T"], meta["G_TOT"]
    NG_t, G_off = meta["NG_t"], meta["G_off"]

    dt = mybir.dt
    nc = bacc.Bacc("TRN2", target_bir_lowering=False, debug=False,
                   enable_asserts=False, num_devices=N_CORES,
                   num_swdge_queues=4)

    stream = nc.dram_tensor("stream", [P, G_TOT, F], dt.float8e3, kind="ExternalInput").ap()
    sel = nc.dram_tensor("sel", [P, G_TOT * P], dt.float8e3, kind="ExternalInput").ap()
    dis_t = nc.dram_tensor("dis_t", [P, T], dt.float32, kind="ExternalInput").ap()
    invdis = nc.dram_tensor("invdis", [1, T * P], dt.bfloat16, kind="ExternalInput").ap()
    xsoT = nc.dram_tensor("xsoT", [P, T, KC * P], dt.bfloat16, kind="ExternalInput").ap()
    wp = nc.dram_tensor("wp", [P, KC, F_OUT], dt.bfloat16, kind="ExternalInput").ap()
    bp = nc.dram_tensor("bp", [1, F_OUT], dt.bfloat16, kind="ExternalInput").ap()
    out = nc.dram_tensor("out", [P, T, F_OUT], dt.bfloat16, kind="ExternalOutput").ap()

    max_bw = max(G_off[min(t0 + TB, T)] - G_off[t0] for t0 in range(0, T, TB))

    with tile.TileContext(nc) as tc:
        with (
            tc.tile_pool(name="const", bufs=1) as cpool,
            tc.tile_pool(name="gbuf", bufs=2) as gpool,
            tc.tile_pool(name="sel8", bufs=2) as s8pool,
            tc.tile_pool(name="xso", bufs=2) as xpool,
            tc.tile_pool(name="aggT", bufs=3) as aggpool,
            tc.tile_pool(name="outsb", bufs=2) as opool,
            tc.tile_pool(name="psA", bufs=3, space="PSUM") as psA,
            tc.tile_pool(name="psB", bufs=3, space="PSUM") as psB,
        ):
            # resident constants
            dis_sb = cpool.tile([P, T], dt.float32, tag="dis")
            nc.sync.dma_start(dis_sb[:], dis_t[:])
            invdis_sb = cpool.tile([1, T * P], dt.bfloat16, tag="invdis")
            nc.sync.dma_start(invdis_sb[:], invdis[:])
            wp_sb = cpool.tile([P, KC, F_OUT], dt.bfloat16, tag="wp")
            nc.sync.dma_start(wp_sb[:], wp[:])
            bp_sb = cpool.tile([1, F_OUT], dt.bfloat16, tag="bp")
            nc.sync.dma_start(bp_sb[:], bp[:])

            for t0 in range(0, T, TB):
                t1 = min(t0 + TB, T)
                nb_t = t1 - t0
                sg0, sg1 = G_off[t0], G_off[t1]

                g_sb = gpool.tile([P, max_bw, F], dt.float8e3, tag="g")
                # first batch split per tile so PE starts sooner
                if t0 == 0:
                    for t in range(t0, t1):
                        ga, gb = G_off[t] - sg0, G_off[t + 1] - sg0
                        nc.sync.dma_start(g_sb[:, ga:gb, :],
                                          stream[:, sg0 + ga:sg0 + gb, :])
                else:
                    nc.sync.dma_start(g_sb[:, :sg1 - sg0, :],
                                      stream[:, sg0:sg1, :])
                self8_sb = s8pool.tile([P, max_bw * P], dt.float8e3, tag="sel8")
                nc.sync.dma_start(self8_sb[:, :(sg1 - sg0) * P],
                                  sel[:, sg0 * P:sg1 * P])
                xso_sb = xpool.tile([P, TB, KC * P], dt.bfloat16, tag="xso")
                nc.sync.dma_start(xso_sb[:, :nb_t, :], xsoT[:, t0:t1, :])
                out_blk = opool.tile([P, TB, F_OUT], dt.bfloat16, tag="out_sb")

                for t in range(t0, t1):
                    ng = NG_t[t]
                    goff = G_off[t] - sg0

                    # selection matmuls: aggT[fchunk, dst] += G_chunk^T @ selR
                    aggT_ps = psA.tile([P, F], dt.float32, tag="aggT_ps")
                    for g in range(ng):
                        for c in range(KC):
                            nc.tensor.matmul(
                                aggT_ps[:, c * P:(c + 1) * P],
                                lhsT=g_sb[:, goff + g, c * P:(c + 1) * P],
                                rhs=self8_sb[:, (goff + g) * P:(goff + g + 1) * P],
                                start=(g == 0 and c == 0),
                                stop=(g == ng - 1 and c == KC - 1),
                                skip_group_check=True,
                            )

                    # PSUM -> SBUF copy with the self-loop term fused in
                    aggT_sb = aggpool.tile([P, F], dt.bfloat16, tag="aggT_sb")
                    nc.vector.tensor_tensor(
                        aggT_sb[:],
                        aggT_ps[:],
                        xso_sb[:, t - t0, :],
                        mybir.AluOpType.add,
                    )

                    # transform GEMM + K=1 bias row (bias pre-scaled by 1/dis)
                    out_ps = psB.tile([P, F_OUT], dt.float32, tag="out_ps")
                    for c in range(KC):
                        nc.tensor.matmul(
                            out_ps[:],
                            lhsT=aggT_sb[:, c * P:(c + 1) * P],
                            rhs=wp_sb[:, c, :],
                            start=(c == 0),
                            stop=False,
                        )
                    nc.tensor.matmul(
                        out_ps[:],
                        lhsT=invdis_sb[:1, t * P:(t + 1) * P],
                        rhs=bp_sb[:1, :],
                        start=False,
                        stop=True,
                    )

                    nc.scalar.activation(
                        out_blk[:, t - t0, :],
                        out_ps[:],
                        mybir.ActivationFunctionType.Relu,
                        scale=dis_sb[:, t:t + 1],
                    )

                nc.sync.dma_start(out[:, t0:t1, :], out_blk[:, :nb_t, :])

    nc.compile()
    return nc


_CACHE = {}


def _get_program(meta):
    key = (meta["N"], meta["F"], meta["F_OUT"], meta["TOT"], meta["G_TOT"],
           tuple(meta["NG_t"]))
    if key not in _CACHE:
        _CACHE[key] = _build_program(meta)
    return _CACHE[key]


def kernel(x, edge_index, W, b, gamma, beta, running_mean, running_var,
           _want_results_holder=None, _run_kwargs=None):
    meta, in_maps = _prep(x, edge_index, W, b, gamma, beta,
                          running_mean, running_var)
    nc = _get_program(meta)

    from concourse.bass_utils import run_bass_kernel_spmd

    res = run_bass_kernel_spmd(nc, in_maps, core_ids=list(range(N_CORES)),
                               **(_run_kwargs or {}))
    if _want_results_holder is not None:
        _want_results_holder.append((nc, meta, in_maps, res))

    T, F_OUT = meta["T"], meta["F_OUT"]
    node_map = meta["node_map"]
    out = np.empty((meta["N"], F_OUT), dtype=np.float32)
    for k in range(N_CORES):
        tiled = np.asarray(res.results[k]["out"], dtype=np.float32)  # [128, T, F_OUT]
        rows = np.ascontiguousarray(tiled.transpose(1, 0, 2))  # [T, 128, F]
        nm = node_map[k]
        valid = nm >= 0
        out[nm[valid]] = rows[valid]
    return out
